# revision 1
# baseline (speedup 1.0000x reference)
"""DCN (deep & cross network) inference kernel for 8 trn2 NeuronCores.

Strategy
--------
Data-parallel over the batch: each of the 8 cores processes 2048 of the
16384 rows.  The cross network is collapsed algebraically:

    xl_{i+1} = x0 * (xl_i . w_i) + b_i + xl_i   (x0 = x)
    =>  xl_3 = x * (1 + S) + (b0+b1+b2)

with S a per-row scalar computable from u_i = x . w_i plus constants
c_ij = b_i . w_j.  Only xl_3 . w_out[:1024] feeds the output, so the
whole cross network reduces to 4 per-row dot products u0..u3
(u3 = x . w_out[:1024]) and ~15 scalar ops per row; those dots are a
[16384,1024]x[1024,4] sgemm the host does in fp32 (precision matters
there - the u's multiply each other - and it is 6% of total flops).

The device runs the dominant deep-tower compute in feature-major layout
(features on partitions, rows on the free axis), with BatchNorm folded
into the following matmul's weights/bias:

    Z.T [64, N]  = w1.T @ x.T                     (the 2.1 GFLOP matmul)
    r   [64, N]  = relu(Z.T + b1)
    t2  [48, N]  = tanh(W2'.T @ r + b2')    -> returned per core

x and w1 stream as float8 E3M4 (4 mantissa bits - measured 6.9e-3 max
rel err end-to-end vs the 2e-2 budget), halving the HBM traffic of the
fp16 variant; w1 is pre-scaled by 64 so all its values are e3m4-normal
(the 1/64 is folded into W2').  Each block's columns are split across
the two halves of the PE array (col-groups 0-1 / 2-3, same weights) so
the M=64 matmul runs two column streams concurrently; the two result
halves land on PSUM partitions 0:64 / 64:128 and get one fused
bias+relu pass.  mm2 then runs as two plain full-K matmuls whose
stationary operands are zero-padded to select one r half each
(contraction depth is free on the systolic array), avoiding the
nonzero-base-partition row tiling that faults real hardware.  out2
returns as e3m4 (tanh outputs are in [-1,1]; measured 8.9e-3 total).  A burst of dummy warm-up matmuls bridges the PE (and
its HAM activity window) until the first x chunk lands, the first chunk
is a single k-tile so real matmuls start at the ~2.2us DMA-latency
floor, the tanh act-table load is primed at kernel start, and mm2/tanh
is software-pipelined one block behind mm1 so the PE never stalls on
the activation chain.  The final 128-row block shortens the serial
tail, and its output DMA issues from the same engine as its tanh.

The host finishes the tiny third layer t3 = tanh(W3'.T t2 + b3'),
hd = (a3*w_out_h) . t3, the cross-scalar recurrence and the sigmoid
(~90 flops/row on 16k rows, negligible).
"""

import numpy as np
import ml_dtypes

B, D = 16384, 1024
N_CORES = 8
ROWS = B // N_CORES          # rows per core
BS = 512                     # max matmul free-dim block
# uneven blocks: the small final block shortens the end-of-kernel serial tail
BLOCKS = [512, 512, 512, 384, 128]
# k-tile spans of each block's DMA chunks: near-line-rate 256-512KB
# transfers, with block 0 split in two so the PE's first real matmul
# starts one DMA-latency (~2.5us) in.  6 total x DMAs keeps the Sync
# ring within the 8 DMA semaphore lanes.
CHUNKS = [[4, 4], [8], [8], [8], [8]]
NBLK = len(BLOCKS)
KT = D // 128                # number of 128-feature contraction tiles
NW = 64                      # tower width
N_WARMUP = 9                 # dummy matmuls to bridge until the first x chunk
WARM_N = 320                 # warm-up matmul free dim
EPS = 1e-3
W1_SCALE = 64.0              # w1 pre-scale so e3m4 stays in normal range

# (offset, size, chunk_flat_offset, chunk_kspans) per block; x is packed
# chunk-contiguous on the host so every chunk DMA is one contiguous region.
_BLK = []
_off = 0
_flat = 0
for _bs, _cks in zip(BLOCKS, CHUNKS):
    assert sum(_cks) == KT
    _BLK.append((_off, _bs, _flat, _cks))
    _off += _bs
    _flat += KT * 128 * _bs
XT_ELEMS = _flat             # == D * ROWS

# const layout inside the fused f16 weight tensor [128, CW]:
#   cols 0:48    -> [W2'; 0]  (W2'/64 on partitions 0:64, zeros below)
#   cols 48:96   -> [0; W2']  (zeros on top, W2'/64 on partitions 64:128)
#   cols 96:98   -> b1*64 duplicated on both halves (f32 bit-pairs)
#   cols 98:100  -> b2' on partitions 0:48 (f32 bit-pairs)
_W2A_OFF = 0
_W2B_OFF = 48
_B1_OFF = 96
_B2_OFF = 98
CW = 100

_STATE: dict = {}


def _build_bass():
    import concourse.bacc as bacc
    import concourse.bass as bass
    import concourse.mybir as mybir
    import concourse.tile as tile

    f32 = mybir.dt.float32
    f16 = mybir.dt.float16
    f8 = mybir.dt.float8e3
    AFT = mybir.ActivationFunctionType

    nc = bacc.Bacc("TRN2", target_bir_lowering=False, debug=False)

    xt = nc.dram_tensor("xt", [XT_ELEMS], f8, kind="ExternalInput")
    w8 = nc.dram_tensor("w8", [128, KT * NW], f8, kind="ExternalInput")
    wts = nc.dram_tensor("wts", [128, CW], f16, kind="ExternalInput")
    out2 = nc.dram_tensor("out2", [48, ROWS], f8, kind="ExternalOutput")

    with tile.TileContext(nc) as tc:
        with (
            tc.tile_pool(name="const", bufs=1) as cpool,
            tc.tile_pool(name="xin", bufs=16) as xpool,
            tc.tile_pool(name="act", bufs=4) as apool,
            tc.tile_pool(name="pz", bufs=3, space=bass.MemorySpace.PSUM) as pz,
            tc.tile_pool(name="p2", bufs=2, space=bass.MemorySpace.PSUM) as p2,
            tc.tile_pool(name="pw", bufs=1, space=bass.MemorySpace.PSUM) as pw,
        ):
            w8_t = cpool.tile([128, KT * NW], f8)
            nc.scalar.dma_start(w8_t[:], w8[:])
            w_t = cpool.tile([128, CW], f16)
            nc.scalar.dma_start(w_t[:], wts[:])

            W2A = w_t[:, _W2A_OFF:_W2A_OFF + 48]      # [W2'; 0]
            W2B = w_t[:, _W2B_OFF:_W2B_OFF + 48]      # [0; W2']
            B1 = w_t[:, _B1_OFF:_B1_OFF + 2].bitcast(f32)          # [128,1]
            B2 = w_t[0:48, _B2_OFF:_B2_OFF + 2].bitcast(f32)

            def wk(k):
                return w8_t[:, k * NW:(k + 1) * NW]

            # prime the tanh act table while the first DMAs are in flight
            zeros = cpool.tile([128, WARM_N], f16)
            nc.vector.memset(zeros[:], 0.0)
            tprime = apool.tile([48, 1], f16, tag="tp")
            nc.scalar.activation(tprime[:], zeros[0:48, 0:1], AFT.Tanh, bias=0.0)

            # PE warm-up: dummy matmuls on a zeroed tile (no DMA dependency,
            # so they start right after the preamble) to keep the PE busy -
            # and the HAM activity window ticking - until the first x chunk
            # lands; the real matmul stream then continues the busy streak.
            wm = pw.tile([NW, WARM_N], f32)
            for _ in range(N_WARMUP):
                nc.tensor.matmul(wm[:], zeros[:, 0:NW], zeros[:],
                                 start=True, stop=True)

            xt_f = xt.ap()  # flat fp8, chunk-contiguous host packing

            rs: dict = {}

            def tower2(i):
                # mm2 + tanh for block i (relu(i) finished a block ago, so
                # the PE never stalls on the activation chain).  The two r
                # halves live on partitions 0:64 / 64:128; each mm2 selects
                # one half via its zero-padded stationary operand.
                r, off, bs, h = rs[i]
                z2 = p2.tile([48, bs], f32, tag="z2")
                nc.tensor.matmul(z2[:, 0:h], W2A, r[:], start=True,
                                 stop=True, skip_group_check=True)
                nc.tensor.matmul(z2[:, h:bs], W2B, r[:], start=True,
                                 stop=True, skip_group_check=True)
                t2 = apool.tile([48, bs], f8, tag="t2")
                nc.scalar.activation(t2[:], z2[:], AFT.Tanh, bias=B2)
                if i == NBLK - 1:
                    # final block: issue from the same engine as the tanh
                    # (no cross-engine hop) on the lower-latency HWDGE path
                    nc.scalar.dma_start(out2[:, off:off + bs], t2[:])
                else:
                    nc.gpsimd.dma_start(out2[:, off:off + bs], t2[:])

            for b, (off, bs, flat, cks) in enumerate(_BLK):
                # stream the block in k-tile chunks so the PE starts as
                # soon as the first chunk lands and DMA never stalls
                chunks = []   # (first_k, kspan, tile)
                pos = flat
                k0 = 0
                for ck in cks:
                    xc = xpool.tile([128, ck, bs], f8, tag="xc")
                    csz = 128 * ck * bs
                    src = xt_f[pos: pos + csz]
                    nc.sync.dma_start(
                        xc[:], src.rearrange("(p k n) -> p k n", p=128, k=ck))
                    chunks.append((k0, ck, xc))
                    pos += csz
                    k0 += ck

                def chunk_ap(k):
                    for k0, ck, xc in chunks:
                        if k0 <= k < k0 + ck:
                            return xc[:, k - k0, :]
                    raise AssertionError

                # last block: its mm1 is gated on the final x chunk
                # anyway, so drain the previous block's mm2/tanh/out-DMA
                # chain first - it is the kernel's serial tail otherwise
                if b == NBLK - 1:
                    tower2(b - 1)

                # column-split mm1: halves of the block stream through the
                # two column halves of the PE array with the same weights.
                # The two accumulation series live on disjoint partition
                # halves of one PSUM bank; the sim's zero-region group check
                # doesn't model partition bases, so skip it.
                h = bs // 2
                zt = pz.tile([128, h], f32, tag="zt")
                for k in range(KT):
                    ca = chunk_ap(k)
                    nc.tensor.matmul(
                        zt[0:64, :], wk(k), ca[:, 0:h],
                        start=(k == 0), stop=(k == KT - 1),
                        tile_position=(0, 0), skip_group_check=True,
                    )
                    nc.tensor.matmul(
                        zt[64:128, :], wk(k), ca[:, h:bs],
                        start=(k == 0), stop=(k == KT - 1),
                        tile_position=(0, 64), skip_group_check=True,
                    )

                r = apool.tile([128, h], f16, tag="r")
                nc.vector.tensor_scalar(
                    r[:], zt[:], B1, 0.0,
                    mybir.AluOpType.add, mybir.AluOpType.max,
                )
                rs[b] = (r, off, bs, h)

                if 1 <= b < NBLK - 1:
                    tower2(b - 1)

            tower2(NBLK - 1)

    nc.compile()
    return nc


def _get_nc():
    if "nc" not in _STATE:
        _STATE["nc"] = _build_bass()
    return _STATE["nc"]


def _prep(inputs):
    """Host-side folding of the tiny weights + the fp32 u-sgemm."""
    f32 = np.float32
    x = np.asarray(inputs["x"], f32)
    cw = np.asarray(inputs["cross_w"], f32)
    cb = np.asarray(inputs["cross_b"], f32)
    w1 = np.asarray(inputs["w1"], f32)
    b1 = np.asarray(inputs["b1"], f32)
    w2 = np.asarray(inputs["w2"], f32)
    b2 = np.asarray(inputs["b2"], f32)
    w3 = np.asarray(inputs["w3"], f32)
    b3 = np.asarray(inputs["b3"], f32)
    w_out = np.asarray(inputs["w_out"], f32)
    b_out = np.asarray(inputs["b_out"], f32)

    def bn_fold(g, be, m, v):
        a = (np.asarray(g, np.float64) / np.sqrt(np.asarray(v, np.float64) + EPS))
        c = np.asarray(be, np.float64) - a * np.asarray(m, np.float64)
        return a, c

    a1, c1 = bn_fold(inputs["gamma1"], inputs["beta1"], inputs["mean1"], inputs["var1"])
    a2, c2 = bn_fold(inputs["gamma2"], inputs["beta2"], inputs["mean2"], inputs["var2"])
    a3, c3 = bn_fold(inputs["gamma3"], inputs["beta3"], inputs["mean3"], inputs["var3"])

    w_out_x = w_out[:D, 0]
    w_out_h = w_out[D:, 0]

    # device computes z*64 (w1 scaled); fold the 1/64 into W2'
    W2p = (a1[:, None] * w2 / W1_SCALE).astype(f32)       # [64, 48]
    b2p = (c1 @ w2 + b2).astype(f32)                      # [48]
    W3p = (a2[:, None] * w3).astype(f32)                  # [48, 24]
    b3p = (c2 @ w3 + b3).astype(f32)                      # [24]
    wh = (a3 * w_out_h).astype(f32)                       # [24]
    ch = float(c3 @ w_out_h)

    c01 = float(cb[0] @ cw[1])
    c02 = float(cb[0] @ cw[2])
    c12 = float(cb[1] @ cw[2])
    c3s = float(cb.sum(axis=0) @ w_out_x)

    # the 4 cross dot products, exact fp32 on host (6% of total flops)
    Wc = np.stack([cw[0], cw[1], cw[2], w_out_x], axis=1).astype(f32)   # [D, 4]
    U = x @ Wc                                                          # [B, 4]

    # device-side const tensors
    w8 = (w1 * W1_SCALE).astype(ml_dtypes.float8_e3m4).reshape(
        KT, 128, NW).transpose(1, 0, 2).reshape(128, -1)                # [128, KT*64]

    wts = np.zeros((128, CW), np.float16)
    wts[0:64, _W2A_OFF:_W2A_OFF + 48] = W2p.astype(np.float16)
    wts[64:128, _W2B_OFF:_W2B_OFF + 48] = W2p.astype(np.float16)
    wts32 = wts.view(np.float32)
    wts32[:, _B1_OFF // 2] = np.concatenate([b1, b1]) * W1_SCALE
    wts32[0:48, _B2_OFF // 2] = b2p

    consts = dict(c01=c01, c02=c02, c12=c12, c3s=c3s, ch=ch,
                  b_out=float(b_out[0]), wh=wh, U=U, W3p=W3p, b3p=b3p)
    return x, w8, wts, consts


def _combine(t2_all, consts):
    """t2_all: [48, B] device tower output -> final sigmoid output [B, 1].

    The host finishes the tiny third layer (16k x 48 x 24 sgemm + tanh),
    the cross-scalar recurrence and the sigmoid."""
    t3 = np.tanh(consts["W3p"].T @ t2_all.astype(np.float32)
                 + consts["b3p"][:, None])                               # [24, B]
    hd = consts["wh"].astype(np.float64) @ t3.astype(np.float64)         # [B]
    U = consts["U"].astype(np.float64)
    u0, u1, u2, u3 = U[:, 0], U[:, 1], U[:, 2], U[:, 3]
    oneS = ((1.0 + u0) * (1.0 + u1) + consts["c01"]) * (1.0 + u2) \
        + consts["c02"] + consts["c12"]
    lin = oneS * u3 + consts["c3s"] + hd + consts["ch"] + consts["b_out"]
    y = 1.0 / (1.0 + np.exp(-lin))
    return y.reshape(-1, 1).astype(np.float32)


def _pack_core(x8c):
    """x8c: [ROWS, KT, 128] e3m4 for one core -> flat chunk-packed stream."""
    parts = []
    for off, bs, _, cks in _BLK:
        blk = x8c[off:off + bs]              # [bs, KT, 128]
        k0 = 0
        for ck in cks:
            parts.append(
                blk[:, k0:k0 + ck, :].transpose(2, 1, 0).ravel())
            k0 += ck
    return np.concatenate(parts)


def _run(inputs, trace=False, **spmd_kwargs):
    from concourse.bass_utils import run_bass_kernel_spmd

    x, w8, wts, consts = _prep(inputs)
    nc = _get_nc()

    x8 = x.astype(ml_dtypes.float8_e3m4).reshape(N_CORES, ROWS, KT, 128)
    in_maps = [
        {"xt": _pack_core(x8[c]), "w8": w8, "wts": wts}
        for c in range(N_CORES)
    ]

    res = run_bass_kernel_spmd(
        nc, in_maps, core_ids=list(range(N_CORES)), trace=trace, **spmd_kwargs
    )
    t2_all = np.concatenate([r["out2"] for r in res.results], axis=1)  # [48, B]
    return _combine(t2_all, consts), res


def kernel(**inputs) -> np.ndarray:
    y, _ = _run(inputs, trace=False)
    return y

